# revision 5
# baseline (speedup 1.0000x reference)
"""Trainium2 Bass kernel for nn_Net_420906795534 (GNN: 3x GraphConv + TopKPooling + readout + MLP).

Sharding: data-parallel over graphs - 8 graphs per NeuronCore x 8 cores.
Host does index-only preprocessing: per-graph dense adjacency count
matrices and layout reshapes. All float compute runs on device.

Device design (fp32r everywhere on the PE so matmuls run at 1 cyc/row
instead of fp32's 4, while staying within ~1e-2 of the fp32 reference
on hardware; fp16/bf16 h was tried and fails on HW, where the ACT
tanh table + reduced h precision flips the top-k selection):
  - conv: agg_T[f,d] = sum_c h_nm_c(f32r).T @ A_c(f32r), streamed as
    512-col PSUM bank tiles from a rotating 5-buffer pool; the same
    weight loads also stream a ones column (previous layer's
    mean-readout sum).
  - linear: W(f32r).T @ [aggT | h'T](f32r), relu on ACT -> fp32 h_T;
    scores in pure fp32 (selection exactness).
  - topk: exact jax.lax.top_k replication via lexicographic cascade of
    gpsimd kth_largest rank extractions (ties at tanh saturation +-1
    broken by previous layers' scores, then node index), with fused
    threshold-compare ops (scalar_tensor_tensor with [P,1] threshold).
  - post: PE transposes to node-major; DVE produces scaled h (f32r,
    next conv lhsT) and the masked tensor for the max readout; the
    scaled h is transposed back (f32r, 1.5 cyc/row) for the next
    layer's root term.
  - ILV=3 graph chains interleaved at sub-layer stage granularity with
    staggered starts so concurrent chains occupy different engines;
    per-graph A is DMA'd in 8 chunks for early compute start.
"""
import sys
sys.path.insert(0, '/opt/trn_rl_repo')
import math
import os
import numpy as np
import ml_dtypes

B_GRAPHS, N, DEG = 64, 1024, 16
IN_F, HID = 20, 128
G_PER_CORE = 8
N_CORES = 8
P = 128
NCH = N // P  # 8 node chunks per graph
XSAT = np.float32(7.998811721801758)  # XLA-cpu f32 tanh saturation cutoff
K1, K2, K3 = 820, 656, 525           # ceil(0.8*n) chain
NDROP = {1: N - K1, 2: K1 - K2, 3: K2 - K3}      # 204, 164, 131
NVALID = {1: N, 2: K1, 3: K2}
KKEEP = {1: K1, 2: K2, 3: K3}
ILV = int(os.environ.get('K_ILV', '3'))  # 3 graph chains in flight
STAGGER = int(os.environ.get('K_STAGGER', '8'))  # stage offset between chains


def _quantile_for_rank(rank_m2: int, n_valid: int) -> float:
    """Return q so kth_largest's k_adj == rank_m2 exactly (frac irrelevant:
    we read out[1] = desc[k_adj+1])."""
    lo = int(math.ceil(rank_m2 * (1 << 32) / (n_valid - 1)))
    hi = int(math.ceil((rank_m2 + 1) * (1 << 32) / (n_valid - 1))) - 1
    omq = (lo + hi) // 2
    assert (omq * (n_valid - 1)) >> 32 == rank_m2
    return 1.0 - omq / (1 << 32)


def build_program():
    import concourse.bacc as bacc
    import concourse.mybir as mybir
    import concourse.tile as tile
    from concourse.masks import make_identity

    f32 = mybir.dt.float32
    f32r = mybir.dt.float32r
    fp16 = mybir.dt.float16
    fp8 = mybir.dt.float8e4
    i32 = mybir.dt.int32
    AF = mybir.ActivationFunctionType
    ALU = mybir.AluOpType
    AX = mybir.AxisListType

    nc = bacc.Bacc("TRN2", target_bir_lowering=False, debug=False,
                   num_devices=N_CORES)

    # ---------------- DRAM I/O ----------------
    d_x = nc.dram_tensor("x_nm", [G_PER_CORE, P, NCH * IN_F], f32, kind="ExternalInput")
    d_A = nc.dram_tensor("A_sd", [G_PER_CORE, P, NCH * N], f32r, kind="ExternalInput")
    d_w = {}
    for l, infl in ((1, IN_F), (2, HID), (3, HID)):
        d_w[f"W_rel{l}"] = nc.dram_tensor(f"W_rel{l}", [infl, HID], f32, kind="ExternalInput")
        d_w[f"W_root{l}"] = nc.dram_tensor(f"W_root{l}", [infl, HID], f32, kind="ExternalInput")
        d_w[f"b_rel{l}"] = nc.dram_tensor(f"b_rel{l}", [HID, 1], f32, kind="ExternalInput")
        d_w[f"w_pool{l}"] = nc.dram_tensor(f"w_pool{l}", [HID, 1], f32, kind="ExternalInput")
    d_w["W_lin1a"] = nc.dram_tensor("W_lin1a", [HID, HID], f32, kind="ExternalInput")
    d_w["W_lin1b"] = nc.dram_tensor("W_lin1b", [HID, HID], f32, kind="ExternalInput")
    d_w["b_lin1"] = nc.dram_tensor("b_lin1", [HID, 1], f32, kind="ExternalInput")
    d_w["W_lin2"] = nc.dram_tensor("W_lin2", [HID, 64], f32, kind="ExternalInput")
    d_w["b_lin2"] = nc.dram_tensor("b_lin2", [64, 1], f32, kind="ExternalInput")
    d_w["W_lin3"] = nc.dram_tensor("W_lin3", [64, 1], f32, kind="ExternalInput")
    d_w["b_lin3"] = nc.dram_tensor("b_lin3", [1, 1], f32, kind="ExternalInput")
    d_out = nc.dram_tensor("out", [1, G_PER_CORE], f32, kind="ExternalOutput")

    with tile.TileContext(nc) as tc:
        with (
            tc.tile_pool(name="const", bufs=1) as cpool,
            tc.tile_pool(name="apool", bufs=ILV) as apool,
            tc.tile_pool(name="hpool", bufs=3) as hpool,
            tc.tile_pool(name="small", bufs=3) as spool,
            tc.tile_pool(name="tiny", bufs=6) as tpool,
            # PSUM 8 banks: 5 x [.,512] rotating + per-chunk transpose
            # tiles (2) + small (1)
            tc.tile_pool(name="psU5", bufs=5, space="PSUM") as psA,
            tc.tile_pool(name="psChunk", bufs=2, space="PSUM") as psC,
            tc.tile_pool(name="psSmall", bufs=1, space="PSUM") as psS,
        ):
            # ---------- constants / weights ----------
            ident = cpool.tile([P, P], f32)
            make_identity(nc, ident[:])
            ident_r = cpool.tile([P, P], f32r)
            nc.vector.tensor_copy(ident_r[:], ident[:])
            ones_f = cpool.tile([P, 2], f32)
            nc.vector.memset(ones_f[:], 1.0)
            ones_r = cpool.tile([P, 2], f32r)
            nc.vector.tensor_copy(ones_r[:], ones_f[:])
            idxb = cpool.tile([P, NCH], f32)
            idxb_i = cpool.tile([P, NCH], i32)
            nc.gpsimd.iota(idxb_i[:], pattern=[[128, NCH]], base=0, channel_multiplier=1)
            nc.vector.tensor_copy(idxb[:], idxb_i[:])

            w_t = {}
            for name, dd in d_w.items():
                t = cpool.tile(list(dd.shape), f32, tag=name)
                # ACT hwdge queue: keeps the SP queue free for x/A loads
                nc.scalar.dma_start(out=t[:], in_=dd[:])
                w_t[name] = t

            w_r = {}
            for l in (1, 2, 3):
                for kind in ("W_rel", "W_root"):
                    rt = cpool.tile(list(d_w[f"{kind}{l}"].shape), f32r,
                                    tag=f"{kind}{l}r", name=f"{kind}{l}r")
                    nc.vector.tensor_copy(rt[:], w_t[f"{kind}{l}"][:])
                    w_r[f"{kind}{l}"] = rt

            # invnorm_l = 1/||w_pool_l|| replicated [P,1]
            invnorm = {}
            for l in (1, 2, 3):
                pnw = psS.tile([1, 1], f32, tag="s")
                nc.tensor.matmul(pnw[:], lhsT=w_t[f"w_pool{l}"][:], rhs=w_t[f"w_pool{l}"][:],
                                 start=True, stop=True)
                nrm = tpool.tile([1, 1], f32, tag="nrm")
                nc.scalar.activation(nrm[:], pnw[:], AF.Sqrt)
                inv = tpool.tile([1, 1], f32, tag="inv")
                nc.vector.reciprocal(inv[:], nrm[:])
                invr = cpool.tile([P, 1], f32, tag=f"invn{l}")
                nc.gpsimd.partition_broadcast(invr[:], inv[:], channels=P)
                invnorm[l] = invr

            # global readout accumulators [feat, graph]
            zmax = cpool.tile([P, G_PER_CORE], f32)
            zmean = cpool.tile([P, G_PER_CORE], f32)
            nc.vector.memset(zmax[:], 0.0)
            nc.vector.memset(zmean[:], 0.0)

            BIG = 1e20
            INVALID = -1e30

            def graph_chain(g):
                s_id = g % ILV
                # ---------- load graph ----------
                t_x = spool.tile([P, NCH * IN_F], f32, tag="x")
                nc.sync.dma_start(out=t_x[:], in_=d_x[g])
                t_xr = spool.tile([P, NCH * IN_F], f32r, tag="xr")
                nc.vector.tensor_copy(t_xr[:], t_x[:])
                t_A = apool.tile([P, NCH * N], f32r, tag="A")
                for c in range(NCH):
                    nc.sync.dma_start(out=t_A[:, c * N:(c + 1) * N],
                                      in_=d_A[g, :, c * N:(c + 1) * N])
                keep = tpool.tile([P, NCH], f32, tag=f"keep{s_id}", bufs=2)
                nc.vector.memset(keep[:], 1.0)
                ucs = []
                h_nm = None       # fp16 node-major scaled h [P, NCH*HID]
                infl = IN_F

                yield

                for l in (1, 2, 3):
                    nvalid, ndrop, kkeep = NVALID[l], NDROP[l], KKEEP[l]
                    # ---------- conv: agg halves + mean-sum in 1-bank tiles --
                    ah_a = spool.tile([infl, N], f32r, tag=f"aha{s_id}", bufs=1)
                    pa0 = psA.tile([infl, 512], f32, tag="u5")
                    pa1 = psA.tile([infl, 512], f32, tag="u5")
                    if l >= 2:
                        psum_prev = psS.tile([infl, 2], f32, tag="s")
                    for c in range(NCH):
                        if l == 1:
                            lhsT = t_xr[:, c * infl:(c + 1) * infl]
                        else:
                            lhsT = h_nm[:, c * infl:(c + 1) * infl]
                        nc.tensor.matmul(
                            pa0[:], lhsT=lhsT,
                            rhs=t_A[:, c * N: c * N + 512],
                            start=(c == 0), stop=(c == NCH - 1),
                            skip_group_check=True)
                        nc.tensor.matmul(
                            pa1[:], lhsT=lhsT,
                            rhs=t_A[:, c * N + 512: (c + 1) * N],
                            start=(c == 0), stop=(c == NCH - 1),
                            skip_group_check=True)
                        if l >= 2:
                            nc.tensor.matmul(
                                psum_prev[:], lhsT=lhsT, rhs=ones_r[:],
                                start=(c == 0), stop=(c == NCH - 1),
                                skip_group_check=True)
                    if l >= 2:
                        nc.vector.scalar_tensor_tensor(
                            out=zmean[:, g:g + 1], in0=psum_prev[:, 0:1],
                            scalar=1.0 / KKEEP[l - 1], in1=zmean[:, g:g + 1],
                            op0=ALU.mult, op1=ALU.add)
                    nc.scalar.copy(ah_a[:, 0:512], pa0[:])
                    nc.scalar.copy(ah_a[:, 512:1024], pa1[:])
                    # identity pass: this layer's root rhs (h'_{l-1}^T / x^T)
                    ah_t = spool.tile([infl, N], f32r, tag=f"aht{s_id}", bufs=1,
                                      name=f"aht{s_id}")
                    for half in range(2):
                        pT = psA.tile([infl, 512], f32r, tag="u5")
                        for ci in range(4):
                            c = half * 4 + ci
                            if l == 1:
                                lhsT = t_xr[:, c * infl:(c + 1) * infl]
                            else:
                                lhsT = h_nm[:, c * infl:(c + 1) * infl]
                            nc.tensor.transpose(
                                pT[:, ci * P:(ci + 1) * P],
                                lhsT, ident_r[:])
                        nc.scalar.copy(ah_t[:, half * 512:(half + 1) * 512], pT[:])

                    yield

                    # ---------- linear: h_T = relu(Wrel.T@aggT + Wroot.T@h'T + b) ----------
                    hT_new = hpool.tile([HID, N], f32, tag=f"hT{s_id}", bufs=2)
                    for half in range(2):
                        sl = slice(half * 512, (half + 1) * 512)
                        ph = psA.tile([HID, 512], f32, tag="u5")
                        nc.tensor.matmul(ph[:],
                                         lhsT=w_r[f"W_rel{l}"][:],
                                         rhs=ah_a[:, sl],
                                         start=True, stop=False, skip_group_check=True)
                        nc.tensor.matmul(ph[:],
                                         lhsT=w_r[f"W_root{l}"][:],
                                         rhs=ah_t[:, sl],
                                         start=False, stop=True, skip_group_check=True)
                        nc.scalar.activation(hT_new[:, sl], ph[:], AF.Relu,
                                             bias=w_t[f"b_rel{l}"][:, 0:1])

                    # ---------- scores (pure fp32) ----------
                    pz = psS.tile([P, NCH], f32, tag="s")
                    for c in range(NCH):
                        nc.tensor.matmul(
                            pz[:, c:c + 1],
                            lhsT=hT_new[:, c * P:(c + 1) * P],
                            rhs=w_t[f"w_pool{l}"][:],
                            start=(c == 0), stop=(c == NCH - 1), skip_group_check=True)
                    u = tpool.tile([P, NCH], f32, tag="u")
                    nc.scalar.activation(u[:], pz[:], AF.Copy, scale=invnorm[l][:, 0:1])
                    uc = tpool.tile([P, NCH], f32, tag=f"uc{l}_{s_id}", bufs=2)
                    nc.vector.tensor_scalar(out=uc[:], in0=u[:], scalar1=float(XSAT),
                                            scalar2=float(-XSAT), op0=ALU.min, op1=ALU.max)
                    ucs.append(uc)

                    yield

                    # ---------- exact top-k keep mask (lex cascade) ----------
                    comps = [("u", t) for t in reversed(ucs)] + [("i", idxb)]
                    bg = tpool.tile([P, NCH], f32, tag="bg")
                    nc.vector.tensor_scalar(out=bg[:], in0=keep[:], scalar1=float(-INVALID),
                                            scalar2=float(INVALID), op0=ALU.mult, op1=ALU.add)
                    ic = tpool.tile([P, NCH], f32, tag="ic")
                    nc.vector.tensor_copy(ic[:], keep[:])
                    dropped = tpool.tile([P, NCH], f32, tag="dropped")
                    nc.vector.memset(dropped[:], 0.0)
                    q = _quantile_for_rank(ndrop - 2, nvalid)
                    for j, (kind, comp) in enumerate(comps):
                        key = tpool.tile([P, NCH], f32, tag="key")
                        nc.vector.tensor_tensor(out=key[:], in0=comp[:], in1=ic[:], op=ALU.mult)
                        if kind == "u":
                            nc.vector.scalar_tensor_tensor(out=key[:], in0=key[:], scalar=-1.0,
                                                           in1=bg[:], op0=ALU.mult, op1=ALU.add)
                        else:
                            nc.vector.tensor_tensor(out=key[:], in0=key[:], in1=bg[:], op=ALU.add)
                        tv = tpool.tile([1, 2], f32, tag="tv")
                        nc.gpsimd.kth_largest(tv[:], key[:], n_per_lane=NCH, k=ndrop,
                                              quantile=q)
                        vrep = tpool.tile([P, 1], f32, tag="vrep")
                        nc.gpsimd.partition_broadcast(vrep[:], tv[:, 1:2], channels=P)
                        last = (j == len(comps) - 1)
                        nd = tpool.tile([P, NCH], f32, tag="nd")
                        nc.vector.scalar_tensor_tensor(
                            out=nd[:], in0=key[:], scalar=vrep[:, 0:1], in1=ic[:],
                            op0=(ALU.is_ge if last else ALU.is_gt), op1=ALU.mult)
                        nc.vector.tensor_tensor(out=dropped[:], in0=dropped[:], in1=nd[:], op=ALU.add)
                        if not last:
                            ic_new = tpool.tile([P, NCH], f32, tag="ic")
                            nc.vector.scalar_tensor_tensor(
                                out=ic_new[:], in0=key[:], scalar=vrep[:, 0:1], in1=ic[:],
                                op0=ALU.is_equal, op1=ALU.mult)
                            safe = tpool.tile([P, NCH], f32, tag="safe")
                            nc.vector.tensor_tensor(out=safe[:], in0=ic[:], in1=ic_new[:], op=ALU.subtract)
                            nc.vector.tensor_tensor(out=safe[:], in0=safe[:], in1=nd[:], op=ALU.subtract)
                            nc.vector.scalar_tensor_tensor(out=bg[:], in0=nd[:], scalar=float(BIG),
                                                           in1=bg[:], op0=ALU.mult, op1=ALU.add)
                            nc.vector.scalar_tensor_tensor(out=bg[:], in0=safe[:], scalar=float(-BIG),
                                                           in1=bg[:], op0=ALU.mult, op1=ALU.add)
                            ic = ic_new
                    keep_new = tpool.tile([P, NCH], f32, tag=f"keep{s_id}", bufs=2)
                    nc.vector.tensor_tensor(out=keep_new[:], in0=keep[:], in1=dropped[:], op=ALU.subtract)
                    keep = keep_new

                    # ---------- scale + mask prep ----------
                    s = tpool.tile([P, NCH], f32, tag="s")
                    nc.scalar.activation(s[:], u[:], AF.Tanh)
                    sk = tpool.tile([P, NCH], f32, tag="sk")
                    nc.vector.tensor_tensor(out=sk[:], in0=s[:], in1=keep[:], op=ALU.mult)
                    maskadd = tpool.tile([P, NCH], f32, tag="maskadd")
                    nc.vector.tensor_scalar(out=maskadd[:], in0=keep[:], scalar1=float(-INVALID),
                                            scalar2=float(INVALID), op0=ALU.mult, op1=ALU.add)

                    yield

                    # ---------- node-major: PE transpose + scale/mask ----
                    h_nm_new = hpool.tile([P, NCH * HID], f32r, tag=f"hnm{s_id}", bufs=1)
                    hm = hpool.tile([P, NCH * HID], f32, tag=f"hm{s_id}", bufs=1)
                    for c in range(NCH):
                        if c == 4:
                            yield
                        csl = slice(c * HID, (c + 1) * HID)
                        pt = psC.tile([P, P], f32, tag="pt")
                        nc.tensor.transpose(pt[:], hT_new[:, c * P:(c + 1) * P], ident[:])
                        nc.vector.tensor_scalar(
                            out=h_nm_new[:, csl], in0=pt[:],
                            scalar1=sk[:, c:c + 1], scalar2=None, op0=ALU.mult)
                        nc.gpsimd.tensor_tensor(
                            out=hm[:, csl], in0=h_nm_new[:, csl],
                            in1=maskadd[:, c:c + 1].to_broadcast([P, HID]),
                            op=ALU.add)

                    # ---------- max readout (f32 strided) ----------
                    pmax = tpool.tile([P, HID], f32, tag="pmax")
                    nc.vector.tensor_reduce(
                        out=pmax[:], in_=hm[:].rearrange("p (c f) -> p f c", c=NCH),
                        axis=AX.X, op=ALU.max)
                    ptm = psC.tile([P, P], f32, tag="pt")
                    nc.tensor.transpose(ptm[:], pmax[:], ident[:])
                    gmax = tpool.tile([P, 1], f32, tag="gmax")
                    nc.vector.tensor_reduce(out=gmax[:], in_=ptm[:], axis=AX.X, op=ALU.max)
                    nc.gpsimd.tensor_tensor(out=zmax[:, g:g + 1], in0=zmax[:, g:g + 1],
                                             in1=gmax[:], op=ALU.add)

                    h_nm = h_nm_new
                    infl = HID
                    yield

                # layer-3 sum readout
                ps3 = psS.tile([HID, 2], f32, tag="s")
                for c in range(NCH):
                    nc.tensor.matmul(ps3[:], lhsT=h_nm[:, c * HID:(c + 1) * HID],
                                     rhs=ones_r[:], start=(c == 0),
                                     stop=(c == NCH - 1),
                                     skip_group_check=True)
                nc.vector.scalar_tensor_tensor(out=zmean[:, g:g + 1], in0=ps3[:, 0:1],
                                               scalar=1.0 / K3, in1=zmean[:, g:g + 1],
                                               op0=ALU.mult, op1=ALU.add)
                yield

            # interleave ILV graph chains, staggered so concurrent chains sit
            # in different stage types (mixes engines instead of convoying)
            chains = [None] * ILV
            started = 0
            done = 0
            rnd = 0
            while done < G_PER_CORE:
                for i in range(ILV):
                    if (chains[i] is None and started < G_PER_CORE
                            and rnd >= i * STAGGER):
                        chains[i] = graph_chain(started)
                        started += 1
                    if chains[i] is not None:
                        try:
                            next(chains[i])
                        except StopIteration:
                            chains[i] = None
                            done += 1
                rnd += 1

            # ---------------- MLP over all graphs (fp32) ----------------
            pa1m = psS.tile([HID, G_PER_CORE], f32, tag="s")
            nc.tensor.matmul(pa1m[:], lhsT=w_t["W_lin1a"][:],
                             rhs=zmax[:], start=True, stop=False,
                             skip_group_check=True)
            nc.tensor.matmul(pa1m[:], lhsT=w_t["W_lin1b"][:],
                             rhs=zmean[:], start=False, stop=True,
                             skip_group_check=True)
            a1 = spool.tile([HID, G_PER_CORE], f32, tag="a1")
            nc.scalar.activation(a1[:], pa1m[:], AF.Relu, bias=w_t["b_lin1"][:, 0:1])
            pa2 = psS.tile([64, G_PER_CORE], f32, tag="s")
            nc.tensor.matmul(pa2[:], lhsT=w_t["W_lin2"][:],
                             rhs=a1[:], start=True, stop=True)
            a2 = spool.tile([64, G_PER_CORE], f32, tag="a2")
            nc.scalar.activation(a2[:], pa2[:], AF.Relu, bias=w_t["b_lin2"][:, 0:1])
            pa3 = psS.tile([1, G_PER_CORE], f32, tag="s")
            nc.tensor.matmul(pa3[:], lhsT=w_t["W_lin3"][:],
                             rhs=a2[:], start=True, stop=True)
            a3 = spool.tile([1, G_PER_CORE], f32, tag="a3")
            nc.scalar.activation(a3[:], pa3[:], AF.Identity, bias=w_t["b_lin3"][:, 0:1])
            nc.sync.dma_start(out=d_out[:], in_=a3[:])

    nc.compile()
    return nc


def prepare_inputs(inputs):
    """Host index-preprocessing + sharding. Returns per-core input maps."""
    x = np.asarray(inputs["x"], np.float32)
    ei = np.asarray(inputs["edge_index"], np.int64)
    src = ei[0] % N
    dst = ei[1] % N
    gid = ei[0] // N

    maps = []
    for core in range(N_CORES):
        gs = range(core * G_PER_CORE, (core + 1) * G_PER_CORE)
        xs = np.empty((G_PER_CORE, P, NCH * IN_F), np.float32)
        As = np.empty((G_PER_CORE, P, NCH * N), np.float32)
        for i, g in enumerate(gs):
            xg = x[g * N:(g + 1) * N].reshape(NCH, P, IN_F).transpose(1, 0, 2)
            xs[i] = xg.reshape(P, NCH * IN_F)
            m = gid == g
            A = np.zeros((N, N), np.float32)
            np.add.at(A, (src[m], dst[m]), 1.0)
            As[i] = (A.reshape(NCH, P, N).transpose(1, 0, 2)
                      .reshape(P, NCH * N))
        im = {"x_nm": xs, "A_sd": As}
        for l in (1, 2, 3):
            im[f"W_rel{l}"] = np.asarray(inputs[f"W_rel{l}"], np.float32)
            im[f"W_root{l}"] = np.asarray(inputs[f"W_root{l}"], np.float32)
            im[f"b_rel{l}"] = np.asarray(inputs[f"b_rel{l}"], np.float32).reshape(HID, 1)
            im[f"w_pool{l}"] = np.asarray(inputs[f"w_pool{l}"], np.float32).reshape(HID, 1)
        W1 = np.asarray(inputs["W_lin1"], np.float32)
        im["W_lin1a"] = np.ascontiguousarray(W1[:HID])
        im["W_lin1b"] = np.ascontiguousarray(W1[HID:])
        im["b_lin1"] = np.asarray(inputs["b_lin1"], np.float32).reshape(HID, 1)
        im["W_lin2"] = np.asarray(inputs["W_lin2"], np.float32)
        im["b_lin2"] = np.asarray(inputs["b_lin2"], np.float32).reshape(64, 1)
        im["W_lin3"] = np.asarray(inputs["W_lin3"], np.float32)
        im["b_lin3"] = np.asarray(inputs["b_lin3"], np.float32).reshape(1, 1)
        maps.append(im)
    return maps


def run_on_device(inputs, trace=False):
    from concourse.bass_utils import run_bass_kernel_spmd
    nc = build_program()
    maps = prepare_inputs(inputs)
    res = run_bass_kernel_spmd(nc, maps, core_ids=list(range(N_CORES)),
                               trace=trace)
    outs = [res.results[c]["out"].reshape(-1) for c in range(N_CORES)]
    full = np.concatenate(outs).astype(np.float32).reshape(B_GRAPHS, 1)
    return full, res


def kernel(**inputs) -> np.ndarray:
    out, _ = run_on_device(inputs)
    return out
